# revision 9
# baseline (speedup 1.0000x reference)
"""3-layer GCN (GCNConv + LayerNorm + ReLU x2, GCNConv) on 8 Trainium2 NeuronCores.

Strategy (node-partitioned, graph-parallel):
  - Nodes are sharded contiguously across the 8 cores (12500 each).
  - Per layer: each core computes the dense transform u = dinv * (h @ W) for its
    own nodes, splits u into a bf16 hi/lo pair (near-fp32 precision), and the
    pairs are AllGather'ed in 4 node-chunks so every core holds the full table.
  - Aggregation (the memory-bound part) runs per destination-block of 128 nodes:
    edges are gathered from the local table with dma_gather (int16 indices,
    4 SWDGE queues for parallel descriptor generation) and segment-summed on the
    TensorEngine via per-column one-hot indicator matmuls (built on the
    VectorEngine with a batched is_equal against an iota tile), accumulating in
    PSUM.  hi/lo sums are recombined in fp32, scaled by dinv[dst], LayerNorm'd
    (bn_stats/bn_aggr) and ReLU'd in a fused scalar-engine activation.
  - Self-loops are ordinary edges; padding slots carry dst=-1 so their indicator
    row is all zeros and they contribute nothing.
"""

import math
import sys
import types
import numpy as np
import ml_dtypes

import concourse.bacc as bacc
import concourse.bass as bass
import concourse.mybir as mybir
from concourse.tile import TileContext
from concourse.vector_clock import ScopedClock
from concourse import bass_utils

F32 = mybir.dt.float32
BF16 = mybir.dt.bfloat16
I16 = mybir.dt.int16
LN_EPS = 1e-5


# ----------------------------------------------------------------------------
# TileContext drain patch: this walrus build rejects >1 sync wait on the
# kernel-tail drain CTRL instruction, so spread the global-clock waits over
# individual sync-engine nops before the drain.
# ----------------------------------------------------------------------------
def _patched_drain_and_barrier(self, tick_clock, wait_clock):
    nc = self.nc
    collector = nc.sync.nop(nofuse=True, hint="drain_wait_split")
    wait_clock.add_sem_waits(collector.ins, ScopedClock({None: tick_clock.global_clock}))
    si = collector.ins.sync_info
    if si is not None and si.on_wait and len(si.on_wait) > 1:
        waits = list(si.on_wait)
        del si.on_wait[1:]
        for w in waits[1:]:
            extra = nc.sync.nop(nofuse=True, hint="drain_wait_split")
            if extra.ins.sync_info is None:
                extra.ins.sync_info = mybir.SyncInfo(on_wait=[w], on_update=[])
            else:
                extra.ins.sync_info.on_wait.append(w)
    nc.sync.drain()
    nc.all_engine_barrier()
    assert self.sems is not None
    popped = nc._tile_sem_poison_stack.pop()
    assert popped is self._sem_poison
    nc.clear_and_free_semaphores(list(self.sems.allocated().values()))
    nc.all_engine_barrier()


TileContext._drain_and_barrier = _patched_drain_and_barrier


# ----------------------------------------------------------------------------
# Configuration
# ----------------------------------------------------------------------------
class Cfg:
    def __init__(self, N=100000, E=1600000, FIN=128, H=128, FOUT=64,
                 NCORES=8, CH=4, GRP=4, KMAXCOL=24):
        self.N, self.E = N, E
        self.FIN, self.H, self.FOUT = FIN, H, FOUT
        self.NC = NCORES
        self.CH = CH              # source chunks (gather table <= 32767 rows each)
        self.GRP = GRP            # dst blocks per group (PSUM tiles in flight)
        self.KMAXCOL = KMAXCOL    # max gather columns per dma_gather call
        assert N % NCORES == 0
        self.OWN = N // NCORES
        assert self.OWN % CH == 0
        self.CR = self.OWN // CH          # rows per core per chunk
        self.CHROWS = self.CR * NCORES    # rows per chunk table
        assert self.CHROWS <= 32767
        self.NB = (self.OWN + 127) // 128  # dst blocks per core
        self.NG = (self.NB + GRP - 1) // GRP


# ----------------------------------------------------------------------------
# Host-side preprocessing
# ----------------------------------------------------------------------------
def _preprocess(cfg, edge_index):
    """Build shared column layout + per-core index/dst arrays."""
    c = cfg
    src = np.asarray(edge_index[0]).astype(np.int64)
    dst = np.asarray(edge_index[1]).astype(np.int64)

    deg = np.bincount(dst, minlength=c.N).astype(np.float32) + 1.0
    dinv = (1.0 / np.sqrt(deg)).astype(np.float32)

    loops = np.arange(c.N, dtype=np.int64)
    s = np.concatenate([src, loops])
    d = np.concatenate([dst, loops])

    ks, rs = np.divmod(s, c.OWN)
    cs, ls = np.divmod(rs, c.CR)
    tloc = (ks * c.CR + ls).astype(np.int64)     # row within chunk table
    kd, rd = np.divmod(d, c.OWN)
    eb = rd // 128                               # dst block within core
    edl = rd % 128                               # dst slot within block

    # per-core per-(block, chunk) edge counts -> shared column layout
    bc = eb * c.CH + cs                          # combined (block, chunk) id
    n_bc = np.zeros((c.NC, c.NB * c.CH), dtype=np.int64)
    for k in range(c.NC):
        m = kd == k
        n_bc[k] = np.bincount(bc[m], minlength=c.NB * c.CH)
    cols_bc = (n_bc.max(axis=0) + 127) // 128    # [NB*CH] shared column counts

    # global column order: (group, chunk, block-within-group)
    order = []                                   # sequence of (b, ch) in layout order
    for g in range(c.NG):
        blocks = range(g * c.GRP, min((g + 1) * c.GRP, c.NB))
        for ch in range(c.CH):
            for b in blocks:
                order.append((b, ch))
    order_idx = {t: i for i, t in enumerate(order)}
    order_pos = np.zeros(c.NB * c.CH, dtype=np.int64)
    for i, (b, ch) in enumerate(order):
        order_pos[b * c.CH + ch] = i
    sizes = np.array([cols_bc[b * c.CH + ch] for (b, ch) in order], dtype=np.int64)
    col_off = np.zeros(len(order) + 1, dtype=np.int64)
    np.cumsum(sizes, out=col_off[1:])
    totcol = int(col_off[-1])

    col_block = np.zeros(totcol, dtype=np.int64)
    col_chunk = np.zeros(totcol, dtype=np.int64)
    for i, (b, ch) in enumerate(order):
        col_block[col_off[i]:col_off[i + 1]] = b
        col_chunk[col_off[i]:col_off[i + 1]] = ch

    first_col = np.full(c.NB, -1, dtype=np.int64)
    last_col = np.zeros(c.NB, dtype=np.int64)
    for q in range(totcol):
        b = col_block[q]
        if first_col[b] < 0:
            first_col[b] = q
        last_col[b] = q

    # gather calls: per (g, ch) consecutive columns, split at KMAXCOL
    calls = [[[] for _ in range(c.CH)] for _ in range(c.NG)]
    for g in range(c.NG):
        blocks = range(g * c.GRP, min((g + 1) * c.GRP, c.NB))
        for ch in range(c.CH):
            i0 = order_idx[(list(blocks)[0], ch)]
            i1 = order_idx[(list(blocks)[-1], ch)]
            q0, q1 = int(col_off[i0]), int(col_off[i1 + 1])
            q = q0
            while q < q1:
                n = min(c.KMAXCOL, q1 - q)
                calls[g][ch].append((q, n))
                q += n

    # per-core padded slot arrays
    idx_all = np.zeros((c.NC, 16, totcol * 8), dtype=np.int16)
    dstf = np.full((c.NC, totcol, 128), -1.0, dtype=np.float32)
    slot_t = np.zeros(totcol * 128, dtype=np.int64)
    for k in range(c.NC):
        m = kd == k
        ebk, csk, tk, dlk = eb[m], cs[m], tloc[m], edl[m]
        okey = order_pos[ebk * c.CH + csk]
        so = np.argsort(okey, kind="stable")
        okey_s, t_s, dl_s = okey[so], tk[so], dlk[so]
        # rank within each (b, ch) run
        counts = np.bincount(okey_s, minlength=len(order))
        run_start_of = np.zeros(len(order) + 1, dtype=np.int64)
        np.cumsum(counts, out=run_start_of[1:])
        within = np.arange(len(okey_s)) - run_start_of[okey_s]
        pos = col_off[okey_s] * 128 + within          # padded slot position
        slot_t[:] = 0
        slot_t[pos] = t_s
        dk = dstf[k].reshape(-1)
        dk[pos] = dl_s.astype(np.float32)
        # wrap indices: slot i of column q -> idx_all[:, i%16, q*8 + i//16]
        st = slot_t.reshape(totcol, 8, 16)            # [q, i//16, i%16]
        idx_all[k] = st.transpose(2, 0, 1).reshape(16, totcol * 8).astype(np.int16)

    meta = dict(totcol=totcol, col_block=col_block, col_chunk=col_chunk,
                first_col=first_col, last_col=last_col, calls=calls)
    return meta, dinv, idx_all, dstf


def _block_row_segments(cfg, b):
    """DRAM row segments of block b split at chunk boundaries: (chunk, row0_in_chunk, slot0, n)."""
    c = cfg
    r0 = b * 128
    r1 = min(r0 + 128, c.OWN)
    segs = []
    r = r0
    while r < r1:
        ch = r // c.CR
        rend = min(r1, (ch + 1) * c.CR)
        segs.append((ch, r - ch * c.CR, r - r0, rend - r))
        r = rend
    return segs


# ----------------------------------------------------------------------------
# Program builder
# ----------------------------------------------------------------------------
def _build_program(cfg, meta, trivial):
    c = cfg
    totcol = meta["totcol"]
    col_block = meta["col_block"]
    first_col = meta["first_col"]
    last_col = meta["last_col"]
    calls = meta["calls"]
    NQ = 4  # SWDGE queues

    nc = bacc.Bacc("TRN2", target_bir_lowering=False, debug=False,
                   num_devices=c.NC, num_swdge_queues=NQ)

    x = nc.dram_tensor("x", (c.OWN, c.FIN), F32, kind="ExternalInput")
    w1 = nc.dram_tensor("w1", (c.FIN, c.H), F32, kind="ExternalInput")
    w2 = nc.dram_tensor("w2", (c.H, c.H), F32, kind="ExternalInput")
    w3 = nc.dram_tensor("w3", (c.H, c.FOUT), F32, kind="ExternalInput")
    idx_all = nc.dram_tensor("idx_all", (128, totcol * 8), I16, kind="ExternalInput")
    dstf = nc.dram_tensor("dstf", (128, totcol), BF16, kind="ExternalInput")
    iota4 = nc.dram_tensor("iota4", (128, 512), BF16, kind="ExternalInput")
    ident = nc.dram_tensor("ident", (128, 128), F32, kind="ExternalInput")
    dinv_cols = nc.dram_tensor("dinv_cols", (128, c.NB), F32, kind="ExternalInput")
    # optional non-trivial affine params (replicated rows)
    aff = {}
    for nm, w in (("b1r", c.H), ("g1r", c.H), ("be1r", c.H),
                  ("b2r", c.H), ("g2r", c.H), ("be2r", c.H), ("b3r", c.FOUT)):
        if not trivial[nm]:
            aff[nm] = nc.dram_tensor(nm, (128, w), F32, kind="ExternalInput")
    y = nc.dram_tensor("y", (c.OWN, c.FOUT), F32, kind="ExternalOutput")

    # exchange buffers (hi/lo bf16 pairs)
    W12, W3p = 2 * c.H, 2 * c.FOUT
    cc_in = [[nc.dram_tensor(f"cc_in{l}_{ch}", (c.CR, W12 if l < 3 else W3p), BF16)
              for ch in range(c.CH)] for l in range(1, 4)]
    cc_out = [[nc.dram_tensor(f"cc_out{l}_{ch}", (c.CHROWS, W12 if l < 3 else W3p),
                              BF16, addr_space="Shared")
               for ch in range(c.CH)] for l in range(1, 4)]

    # AG trigger points: after which group each chunk's contribution rows are done
    def ag_group(ch):
        last_row = (ch + 1) * c.CR - 1
        return (last_row // 128) // c.GRP

    with TileContext(nc) as tc:
        consts = tc.alloc_tile_pool(name="consts", bufs=1)
        xh = tc.alloc_tile_pool(name="xh", bufs=4)
        xtp = tc.alloc_tile_pool(name="xtp", bufs=3)
        tsp = tc.alloc_tile_pool(name="tsp", bufs=4)
        uhl = tc.alloc_tile_pool(name="uhl", bufs=6)
        gp = tc.alloc_tile_pool(name="gp", bufs=4)
        ip = tc.alloc_tile_pool(name="ip", bufs=4)
        lnp = tc.alloc_tile_pool(name="lnp", bufs=6)
        ps_t = tc.alloc_tile_pool(name="ps_t", bufs=2, space="PSUM")
        ps_d = tc.alloc_tile_pool(name="ps_d", bufs=2, space="PSUM")
        ps_a = tc.alloc_tile_pool(name="ps_a", bufs=4, space="PSUM")

        w1_sb = consts.tile([c.FIN, c.H], F32, tag="w1")
        w2_sb = consts.tile([c.H, c.H], F32, tag="w2")
        w3_sb = consts.tile([c.H, c.FOUT], F32, tag="w3")
        idx_sb = consts.tile([128, totcol * 8], I16, tag="idx")
        dstf_sb = consts.tile([128, totcol], BF16, tag="dstf")
        iota4_sb = consts.tile([128, 512], BF16, tag="iota4")
        ident_sb = consts.tile([128, 128], F32, tag="ident")
        dinv_sb = consts.tile([128, c.NB], F32, tag="dinv")
        eps_sb = consts.tile([128, 1], F32, tag="eps")
        nc.sync.dma_start(out=w1_sb[:], in_=w1[:])
        nc.sync.dma_start(out=w2_sb[:], in_=w2[:])
        nc.sync.dma_start(out=w3_sb[:], in_=w3[:])
        nc.sync.dma_start(out=idx_sb[:], in_=idx_all[:])
        nc.sync.dma_start(out=dstf_sb[:], in_=dstf[:])
        nc.sync.dma_start(out=iota4_sb[:], in_=iota4[:])
        nc.sync.dma_start(out=ident_sb[:], in_=ident[:])
        nc.sync.dma_start(out=dinv_sb[:], in_=dinv_cols[:])
        nc.vector.memset(eps_sb[:], LN_EPS)
        aff_sb = {}
        for nm, t in aff.items():
            aff_sb[nm] = consts.tile(list(t.shape), F32, tag=nm, name=nm)
            nc.sync.dma_start(out=aff_sb[nm][:], in_=t[:])

        def dense_and_split(h_sb, layer, b):
            """h_sb [128, H] fp32 -> u = dinv*(h@W); write bf16 hi/lo to cc_in[layer]."""
            wname = (w1_sb, w2_sb, w3_sb)[layer - 1]
            fout = c.H if layer < 3 else c.FOUT
            tp = ps_t.tile([128, 128], F32, tag="tps")
            nc.tensor.transpose(out=tp[:], in_=h_sb[:], identity=ident_sb[:])
            hT = xtp.tile([128, 128], F32, tag="hT")
            nc.scalar.copy(out=hT[:], in_=tp[:])
            dp = ps_d.tile([128, c.H], F32, tag="dps")
            nc.tensor.matmul(dp[:, :fout], lhsT=hT[:], rhs=wname[:], start=True, stop=True)
            t = tsp.tile([128, c.H], F32, tag="tsplit")
            nc.vector.tensor_scalar_mul(out=t[:, :fout], in0=dp[:, :fout],
                                        scalar1=dinv_sb[:, b:b + 1])
            uh = uhl.tile([128, c.H], BF16, tag="uh")
            uhf = tsp.tile([128, c.H], F32, tag="uhf")
            ul = uhl.tile([128, c.H], BF16, tag="ul")
            nc.scalar.copy(out=uh[:, :fout], in_=t[:, :fout])
            nc.scalar.copy(out=uhf[:, :fout], in_=uh[:, :fout])
            nc.vector.tensor_tensor(out=ul[:, :fout], in0=t[:, :fout], in1=uhf[:, :fout],
                                    op=mybir.AluOpType.subtract)
            for (ch, row0, slot0, nrows) in _block_row_segments(c, b):
                tgt = cc_in[layer - 1][ch]
                nc.sync.dma_start(out=tgt[row0:row0 + nrows, :fout],
                                  in_=uh[slot0:slot0 + nrows, :fout])
                nc.sync.dma_start(out=tgt[row0:row0 + nrows, fout:2 * fout],
                                  in_=ul[slot0:slot0 + nrows, :fout])

        def emit_ag(layer):
            done = [False] * c.CH

            def maybe(g):
                for ch in range(c.CH):
                    if not done[ch] and g >= ag_group(ch):
                        done[ch] = True
                        w = cc_in[layer - 1][ch].shape[1]
                        nc.gpsimd.collective_compute(
                            "AllGather", mybir.AluOpType.bypass,
                            replica_groups=[list(range(c.NC))],
                            ins=[cc_in[layer - 1][ch][:]],
                            outs=[cc_out[layer - 1][ch][:]],
                        )
            return maybe

        # ---------------- layer 1 dense ----------------
        ag1 = emit_ag(1)
        for g in range(c.NG):
            for b in range(g * c.GRP, min((g + 1) * c.GRP, c.NB)):
                r0 = b * 128
                nrows = min(128, c.OWN - r0)
                xb = xh.tile([128, c.FIN], F32, tag="xh")
                if nrows < 128:
                    nc.vector.memset(xb[:], 0.0)
                nc.sync.dma_start(out=xb[:nrows, :], in_=x[r0:r0 + nrows, :])
                dense_and_split(xb, 1, b)
            ag1(g)

        # ---------------- aggregation layers ----------------
        def agg_layer(layer):
            """Aggregate from cc_out[layer-1]; layer<3: LN+ReLU then dense(layer+1);
            layer==3: final output."""
            elem = 2 * (c.H if layer < 3 else c.FOUT)
            psw = elem
            psum_tiles = {}
            ag_next = emit_ag(layer + 1) if layer < 3 else None
            for g in range(c.NG):
                blocks = range(g * c.GRP, min((g + 1) * c.GRP, c.NB))
                for ch in range(c.CH):
                    for (q0, ncols) in calls[g][ch]:
                        gt = gp.tile([128, c.KMAXCOL, elem], BF16, tag="gt")
                        nc.gpsimd.dma_gather(
                            gt[:, :ncols, :], cc_out[layer - 1][ch][:],
                            idx_sb[:, q0 * 8:(q0 + ncols) * 8],
                            ncols * 128, ncols * 128, elem,
                            single_packet=False, queue_num=ch % NQ)
                        ind = ip.tile([128, c.KMAXCOL, 128], BF16, tag="ind")
                        for j0 in range(0, ncols, 4):
                            nb_ = min(4, ncols - j0)
                            in0 = iota4_sb[:].rearrange("p (n s) -> p n s", s=128)[:, :nb_, :]
                            sl = dstf_sb[:, q0 + j0:q0 + j0 + nb_]
                            in1 = bass.AP(tensor=sl.tensor, offset=sl.offset,
                                          ap=[list(sl.ap[0]), list(sl.ap[1]), [0, 128]])
                            nc.vector.tensor_tensor(out=ind[:, j0:j0 + nb_, :],
                                                    in0=in0, in1=in1,
                                                    op=mybir.AluOpType.is_equal)
                        for j in range(ncols):
                            q = q0 + j
                            b = int(col_block[q])
                            if b not in psum_tiles:
                                psum_tiles[b] = ps_a.tile([128, W12], F32, tag="aps", name=f"aps_{layer}_{b}")
                            nc.tensor.matmul(
                                psum_tiles[b][:, :psw],
                                lhsT=ind[:, j, :], rhs=gt[:, j, :],
                                start=(q == first_col[b]), stop=(q == last_col[b]))
                # post-process completed blocks of this group
                for b in blocks:
                    ps = psum_tiles.pop(b)
                    fo = c.H if layer < 3 else c.FOUT
                    lo_sb = lnp.tile([128, c.H], F32, tag="lo_sb")
                    nc.scalar.copy(out=lo_sb[:, :fo], in_=ps[:, fo:2 * fo])
                    t0 = lnp.tile([128, c.H], F32, tag="t0")
                    nc.vector.tensor_tensor(out=t0[:, :fo], in0=ps[:, :fo],
                                            in1=lo_sb[:, :fo],
                                            op=mybir.AluOpType.add)
                    if layer < 3:
                        bias_nm, gain_nm, beta_nm = (f"b{layer}r", f"g{layer}r", f"be{layer}r")
                        t = lnp.tile([128, c.H], F32, tag="t")
                        nc.vector.tensor_scalar_mul(out=t[:], in0=t0[:],
                                                    scalar1=dinv_sb[:, b:b + 1])
                        if bias_nm in aff_sb:
                            nc.vector.tensor_tensor(out=t[:], in0=t[:],
                                                    in1=aff_sb[bias_nm][:],
                                                    op=mybir.AluOpType.add)
                        stats = lnp.tile([128, 6], F32, tag="stats")
                        nc.vector.bn_stats(out=stats[:], in_=t[:])
                        mv = lnp.tile([128, 2], F32, tag="mv")
                        nc.vector.bn_aggr(out=mv[:], in_=stats[:])
                        sd = lnp.tile([128, 1], F32, tag="sd")
                        nc.scalar.activation(out=sd[:], in_=mv[:, 1:2],
                                             func=mybir.ActivationFunctionType.Sqrt,
                                             bias=eps_sb[:])
                        rstd = lnp.tile([128, 1], F32, tag="rstd")
                        nc.vector.reciprocal(out=rstd[:], in_=sd[:])
                        nbias = lnp.tile([128, 1], F32, tag="nbias")
                        nc.vector.tensor_scalar(out=nbias[:], in0=mv[:, 0:1],
                                                scalar1=rstd[:], scalar2=-1.0,
                                                op0=mybir.AluOpType.mult,
                                                op1=mybir.AluOpType.mult)
                        h = xh.tile([128, c.H], F32, tag="xh")
                        if gain_nm in aff_sb or beta_nm in aff_sb:
                            hn = lnp.tile([128, c.H], F32, tag="hn")
                            nc.scalar.activation(out=hn[:], in_=t[:],
                                                 func=mybir.ActivationFunctionType.Copy,
                                                 scale=rstd[:], bias=nbias[:])
                            if gain_nm in aff_sb:
                                nc.vector.tensor_tensor(out=hn[:], in0=hn[:],
                                                        in1=aff_sb[gain_nm][:],
                                                        op=mybir.AluOpType.mult)
                            if beta_nm in aff_sb:
                                nc.vector.tensor_tensor(out=hn[:], in0=hn[:],
                                                        in1=aff_sb[beta_nm][:],
                                                        op=mybir.AluOpType.add)
                            nc.scalar.activation(out=h[:], in_=hn[:],
                                                 func=mybir.ActivationFunctionType.Relu)
                        else:
                            nc.scalar.activation(out=h[:], in_=t[:],
                                                 func=mybir.ActivationFunctionType.Relu,
                                                 scale=rstd[:], bias=nbias[:])
                        dense_and_split(h, layer + 1, b)
                    else:
                        o = lnp.tile([128, c.FOUT], F32, tag="o")
                        nc.vector.tensor_scalar_mul(out=o[:], in0=t0[:, :fo],
                                                    scalar1=dinv_sb[:, b:b + 1])
                        if "b3r" in aff_sb:
                            nc.vector.tensor_tensor(out=o[:], in0=o[:],
                                                    in1=aff_sb["b3r"][:],
                                                    op=mybir.AluOpType.add)
                        r0 = b * 128
                        nrows = min(128, c.OWN - r0)
                        nc.sync.dma_start(out=y[r0:r0 + nrows, :], in_=o[:nrows, :])
                if ag_next is not None:
                    ag_next(g)

        agg_layer(1)
        agg_layer(2)
        agg_layer(3)

        for p in (ps_a, ps_d, ps_t, lnp, ip, gp, uhl, tsp, xtp, xh, consts):
            p.release()

    nc.compile()
    return nc


# ----------------------------------------------------------------------------
# Entry points
# ----------------------------------------------------------------------------
_cache = {}


def _prepare(cfg, inputs):
    c = cfg
    key = hash((np.asarray(inputs["edge_index"]).tobytes(),))
    if key in _cache:
        return _cache[key]

    meta, dinv, idx_all, dstf = _preprocess(c, inputs["edge_index"])

    trivial = {
        "b1r": not np.any(inputs["b1"]), "g1r": bool(np.all(inputs["g1"] == 1.0)),
        "be1r": not np.any(inputs["be1"]), "b2r": not np.any(inputs["b2"]),
        "g2r": bool(np.all(inputs["g2"] == 1.0)), "be2r": not np.any(inputs["be2"]),
        "b3r": not np.any(inputs["b3"]),
    }
    nc = _build_program(c, meta, trivial)

    iota4 = np.tile(np.arange(128, dtype=np.float32), 4)[None, :].repeat(128, 0)
    shared = {
        "w1": np.asarray(inputs["W1"], dtype=np.float32),
        "w2": np.asarray(inputs["W2"], dtype=np.float32),
        "w3": np.asarray(inputs["W3"], dtype=np.float32),
        "iota4": iota4.astype(ml_dtypes.bfloat16),
        "ident": np.eye(128, dtype=np.float32),
    }
    for nm, src in (("b1r", "b1"), ("g1r", "g1"), ("be1r", "be1"), ("b2r", "b2"),
                    ("g2r", "g2"), ("be2r", "be2"), ("b3r", "b3")):
        if not trivial[nm]:
            shared[nm] = np.asarray(inputs[src], dtype=np.float32)[None, :].repeat(128, 0).copy()

    x_np = np.asarray(inputs["x"], dtype=np.float32)
    in_maps = []
    for k in range(c.NC):
        dv = dinv[k * c.OWN:(k + 1) * c.OWN]
        dcols = np.zeros((128, c.NB), dtype=np.float32)
        npad = c.NB * 128 - c.OWN
        dvp = np.concatenate([dv, np.ones(npad, dtype=np.float32)])
        dcols[:, :] = dvp.reshape(c.NB, 128).T
        m = dict(shared)
        m["x"] = x_np[k * c.OWN:(k + 1) * c.OWN]
        m["idx_all"] = np.tile(idx_all[k], (8, 1))
        m["dstf"] = np.ascontiguousarray(dstf[k].T).astype(ml_dtypes.bfloat16)
        m["dinv_cols"] = dcols
        in_maps.append(m)

    _cache[key] = (nc, in_maps)
    return nc, in_maps


def _run(cfg, inputs, trace=False):
    nc, in_maps = _prepare(cfg, inputs)
    res = bass_utils.run_bass_kernel_spmd(
        nc, in_maps, core_ids=list(range(cfg.NC)), trace=trace)
    out = np.concatenate([res.results[k]["y"] for k in range(cfg.NC)], axis=0)
    return out, res


def kernel(**inputs):
    cfg = Cfg()
    out, _ = _run(cfg, inputs)
    return out
